# revision 35
# baseline (speedup 1.0000x reference)
"""GAT message-passing kernel for Trainium2, 8 NeuronCores — v2.

Problem (hardcoded): B=4, N=1024, H=F=O=G=128, E=16.
  features = concat([n_features, hidden], -1)            [B,N,256]
  values   = features @ W_m + b_m                        [B,N,128]
  logits   = att1 + att2^T + (e_features@w_ae) + att_g   [B,N,N]
  coefs    = softmax(leaky_relu(logits) + (adj-1)*1e9)
  out      = coefs @ values + features @ W_skip + b_skip

Sharding: 8 cores = (batch b = core//2) x (row half = core%2): each core
owns 512 query rows of one batch; keys unsharded. No collectives.

Design (85 us v1 -> ~76-82 us): memory-bound on the e_features stream;
halve its bytes with fp8 and restructure attention so no engine sits on
the DMA critical path.
  - ef is quantized host-side to fp8 e4m3 with ERROR-FEEDBACK across
    the E dim (descending |w_ae|): each channel's quantization residual
    is carried into the next channel, so the on-chip dot product
    sum_e ef_e*w_e lands at bf16-level accuracy (2.7e-3) at half the
    HBM traffic (8 MiB/core).
  - TRANSPOSED attention: logits are built as lps[key, row] per
    128-key chunk. Softmax normalization moves to a ones-matmul
    (PSUM-accumulated across chunks) and A@V consumes the masked
    coefs^T tile directly as the PE moving operand with V stationary —
    no PE transposes, no PSUM->SBUF copies on ACT.
  - The E-contraction runs in DoubleRow fp8 (2 elems/lane/cycle):
    stationary Wd[(km2,es),j,km'] block-diagonal over 64 keys with 4
    channels (es,j) per s-group; out partitions 0..63 (DR requires full
    128-col footprint), so the pipeline works on two [64,512] PSUM
    tiles per chunk.
  - exp(leaky_relu(x)) = max(exp(x+b), exp(.01x+.01b)): two Exp ACT ops
    (ONE act-table set - Lrelu/Exp alternation costs a 1.3us table load
    per op) with att2+att_g+biases on the per-partition bias port; att1
    rides a rank-1 matmul into PSUM. adj mask multiplies post-exp (DVE).
  - Scheduling (the real battle): early-need constants ride the sync
    HWDGE queue AHEAD of the ef stream (SWDGE receipt is ~2us slower and
    13 queued constants starved the HWDGE DMA semaphore lanes, gating ef
    issue); the rest load via SWDGE (separate lane pool). V/att2 are
    produced just-in-time in-loop sharing fTk stationaries; A@V runs at
    pipeline depth 2; a zero-weight matmul spin + heaters (into the
    accumulating PSUM, adding 0) bridge PE idle so the HAM clock-gate
    holds 2.4 GHz.
  - Output is produced transposed [O, rows] and un-transposed on host.
"""

import os
import numpy as np

B, N, H, F, E, G, O = 4, 1024, 128, 128, 16, 128, 128
DIN = F + H
NCORES = 8
ROWS = N // 2          # query rows per core
KC = N // 128          # key chunks of 128
KG = 2                 # 64-key groups per chunk
NS = 4                 # s-groups (4 channels (es,j) each)
NPAIR = KC // 2        # ef DMA tiles (2 chunks each)

_cache = {}


def _build():
    from contextlib import ExitStack
    import concourse.bacc as bacc
    import concourse.tile as tile
    import concourse.mybir as mybir

    fp32 = mybir.dt.float32
    bf16 = mybir.dt.bfloat16
    f8 = mybir.dt.float8e4
    AF = mybir.ActivationFunctionType
    DR = mybir.MatmulPerfMode.DoubleRow

    nc = bacc.Bacc("TRN2", target_bir_lowering=False, debug=False,
                   num_devices=NCORES)

    # T[p=(km2*2+es), kc, kg, s, j, r] fp8 codes of ef
    T_in = nc.dram_tensor("T", [128, KC, KG, NS, 2, ROWS], f8,
                          kind="ExternalInput")
    Wd_in = nc.dram_tensor("Wd", [128, NS, 2, 64], f8, kind="ExternalInput")
    adjT_in = nc.dram_tensor("adjT", [64, KC, KG, ROWS], bf16,
                             kind="ExternalInput")
    fTk_in = nc.dram_tensor("fTk", [128, 2, N], bf16, kind="ExternalInput")
    fTr_in = nc.dram_tensor("fTr", [128, 2, ROWS], bf16, kind="ExternalInput")
    Wm_in = nc.dram_tensor("Wm", [128, 2, O], bf16, kind="ExternalInput")
    Wsk_in = nc.dram_tensor("Wsk", [128, 2, O], bf16, kind="ExternalInput")
    sm_in = nc.dram_tensor("sm", [128, 136], bf16, kind="ExternalInput")
    out_t = nc.dram_tensor("out", [O, ROWS], fp32, kind="ExternalOutput")

    with tile.TileContext(nc) as tc:
        with ExitStack() as ctx:
            singles = ctx.enter_context(tc.tile_pool(name="singles", bufs=1))
            efp = ctx.enter_context(tc.tile_pool(name="efp", bufs=8))
            work = ctx.enter_context(tc.tile_pool(name="work", bufs=4))
            psL = ctx.enter_context(tc.tile_pool(name="psL", bufs=2,
                                                 space="PSUM"))
            psO = ctx.enter_context(tc.tile_pool(name="psO", bufs=1,
                                                 space="PSUM"))
            psS = ctx.enter_context(tc.tile_pool(name="psS", bufs=1,
                                                 space="PSUM"))
            psR = ctx.enter_context(tc.tile_pool(name="psR", bufs=2,
                                                 space="PSUM"))

            # ---- early constants on sync HWDGE (land ~10us), then ef ---
            smalls = singles.tile([128, 136], bf16)
            nc.sync.dma_start(out=smalls, in_=sm_in.ap())
            w12_sb = smalls[:, 0:4]
            g_sb = smalls[:, 4:5]
            wag_sb = smalls[:, 5:6]
            bsb_sb = smalls[:, 6:7]
            bsk_sb = smalls[0:1, 8:136]
            Wd_sb = singles.tile([128, NS, 2, 64], f8)
            nc.sync.dma_start(out=Wd_sb, in_=Wd_in.ap())
            fTr = singles.tile([128, 2, ROWS], bf16)
            nc.sync.dma_start(out=fTr, in_=fTr_in.ap())

            # ---- ef stream (sync HWDGE queue) --------------------------
            ef_tiles = []
            for p in range(KC):
                t = efp.tile([128, KG, NS, 2, ROWS], f8, tag="ef")
                if p < 2:
                    nc.sync.dma_start(out=t[:, 0], in_=T_in[:, p, 0])
                    nc.sync.dma_start(out=t[:, 1], in_=T_in[:, p, 1])
                else:
                    nc.sync.dma_start(out=t, in_=T_in[:, p])
                ef_tiles.append(t)

            # ---- remaining constants (SWDGE queue) ---------------------
            fTk = singles.tile([128, 2, N], bf16)
            nc.gpsimd.dma_start(out=fTk, in_=fTk_in.ap())
            Wm_sb = singles.tile([128, 2, O], bf16)
            nc.gpsimd.dma_start(out=Wm_sb, in_=Wm_in.ap())
            adjT = singles.tile([64, KC, KG, ROWS], bf16)
            nc.gpsimd.dma_start(out=adjT[:, 0:4], in_=adjT_in[:, 0:4])
            nc.gpsimd.dma_start(out=adjT[:, 4:8], in_=adjT_in[:, 4:8])
            Wsk_sb = singles.tile([128, 2, O], bf16)
            nc.gpsimd.dma_start(out=Wsk_sb, in_=Wsk_in.ap())

            ones_bf = singles.tile([1, 128], bf16)
            nc.vector.memset(ones_bf, 1.0)
            ones512 = singles.tile([1, ROWS], bf16)
            nc.vector.memset(ones512, 1.0)
            ones64c = singles.tile([64, 1], bf16)
            nc.vector.memset(ones64c, 1.0)
            ones_f32 = singles.tile([1, 128], fp32)
            nc.vector.memset(ones_f32, 1.0)
            w0 = singles.tile([128, 128], bf16)
            nc.vector.memset(w0, 0.0)
            w0f8 = singles.tile([128, 128], f8)
            nc.vector.memset(w0f8, 0.0)

            # accumulators (also double as warmup/heater scratch: zero-weight
            # matmuls add 0; the first real av() has start=True which clears)
            outT_ps = psO.tile([128, ROWS], fp32, tag="o")
            s_ps = psS.tile([1, ROWS], fp32, tag="s")

            # PE warmup spin: sustained activity flips HAM to 8/8 early
            for i in range(48):
                nc.tensor.matmul(outT_ps[:, 0:128], w0, w0,
                                 start=(i == 0), stop=False,
                                 tile_position=(0, 0), skip_group_check=True)

            # ---- phase 0: V, att2, att1, skipT, att_g ------------------
            # att_g + sum(biases): sc = g@wag + bs -> bcast [64,1]
            gps = psR.tile([1, 1], fp32, tag="ret")
            nc.tensor.matmul(gps, g_sb, wag_sb, start=True, stop=True)
            bsf = singles.tile([1, 1], fp32)
            nc.vector.tensor_copy(bsf, bsb_sb[0:1, :])
            sc1 = singles.tile([1, 1], fp32)
            nc.vector.tensor_copy(sc1, gps)
            nc.vector.tensor_scalar_add(sc1, sc1, bsf)


            # att1 over this core's rows, + (att_g + biases): [1, ROWS]
            a1ps = psR.tile([1, ROWS], fp32, tag="ret")
            nc.tensor.matmul(a1ps, w12_sb[:, 0:1], fTr[:, 0, :],
                             start=True, stop=False)
            nc.tensor.matmul(a1ps, w12_sb[:, 1:2], fTr[:, 1, :],
                             start=False, stop=True)
            a1f = singles.tile([1, ROWS], fp32)
            nc.vector.tensor_scalar_add(a1f, a1ps, sc1)
            att1r = singles.tile([1, ROWS], bf16)
            nc.vector.tensor_copy(att1r, a1f)

            # ---- main loop over key chunks (software-pipelined) --------
            V_sb = singles.tile([64, KC * KG, O], bf16)
            att2b = singles.tile([64, KC * KG], fp32)
            cT_tiles = {}

            def contract(kc):
                Ekc = ef_tiles[kc]          # [128, KG, NS, 2, ROWS]
                lps = []
                for kg in range(KG):
                    lp = psL.tile([64, ROWS], fp32, tag=f"l{kg}")
                    for s in range(NS):
                        nc.tensor.matmul(lp, Wd_sb[:, s], Ekc[:, kg, s],
                                         start=(s == 0), stop=False,
                                         perf_mode=DR, tile_position=(0, 0))
                    nc.tensor.matmul(lp, ones_bf[:, 0:64], att1r,
                                     start=False, stop=True,
                                     skip_group_check=True)
                    lps.append(lp)
                return lps

            def vmm2(p):
                # V[key64, O] + att2[key64] for 4 half-chunks (2 kc) at once
                vps = psR.tile([64, 4, O], fp32, tag="ret")
                aps = psR.tile([64, 4], fp32, tag="ret")
                for i in range(4):
                    idx = p * 4 + i
                    ks = slice(idx * 64, idx * 64 + 64)
                    nc.tensor.matmul(vps[:, i, :], fTk[:, 0, ks],
                                     Wm_sb[:, 0, :], start=True, stop=False,
                                     skip_group_check=True)
                    nc.tensor.matmul(vps[:, i, :], fTk[:, 1, ks],
                                     Wm_sb[:, 1, :], start=False, stop=True,
                                     skip_group_check=True)
                    nc.tensor.matmul(aps[:, i:i + 1], fTk[:, 0, ks],
                                     w12_sb[:, 2:3], start=True, stop=False,
                                     skip_group_check=True)
                    nc.tensor.matmul(aps[:, i:i + 1], fTk[:, 1, ks],
                                     w12_sb[:, 3:4], start=False, stop=True,
                                     skip_group_check=True)
                nc.vector.tensor_copy(V_sb[:, p * 4:p * 4 + 4, :], vps)
                nc.vector.tensor_copy(att2b[:, p * 4:p * 4 + 4], aps)

            def softmax_mask(kc, lps):
                lrl = work.tile([64, KG, ROWS], fp32, tag="lrl")
                ex = work.tile([64, KG, ROWS], bf16, tag="ex")
                for kg in range(KG):
                    idx = kc * KG + kg
                    nc.scalar.activation(lrl[:, kg, :], lps[kg], AF.Lrelu,
                                         bias=att2b[:, idx:idx + 1],
                                         alpha=0.01)
                nc.scalar.activation(ex, lrl, AF.Exp)
                cT = work.tile([64, KG, ROWS], bf16, tag="cT")
                nc.vector.tensor_mul(cT, ex, adjT[:, kc])
                cT_tiles[kc] = cT

            def av(kc):
                cT = cT_tiles.pop(kc)
                for kg in range(KG):
                    idx = kc * KG + kg
                    first = (kc == 0 and kg == 0)
                    last = (kc == KC - 1 and kg == KG - 1)
                    nc.tensor.matmul(s_ps, ones64c, cT[:, kg, :],
                                     start=first, stop=last,
                                     skip_group_check=True)
                    nc.tensor.matmul(outT_ps, V_sb[:, idx, :], cT[:, kg, :],
                                     start=first, stop=last,
                                     skip_group_check=True)

            for kc in range(KC):
                if kc >= 3:
                    av(kc - 3)
                if kc < 2:
                    vmm2(2 * kc)
                    vmm2(2 * kc + 1)
                if 1 <= kc <= 5:
                    # zero-weight heaters: keep the PE HAM-warm while the
                    # ef stream ramps; numerically add 0 to outT
                    for _ in range(2):
                        nc.tensor.matmul(outT_ps[:, 0:ROWS],
                                         w0f8, ef_tiles[kc - 1][:, 0, 0, 0],
                                         start=False, stop=False,
                                         skip_group_check=True)
                lps = contract(kc)
                softmax_mask(kc, lps)
            av(KC - 3)
            av(KC - 2)
            av(KC - 1)

            # ---- skip^T, normalize, add, store -------------------------
            skp = psR.tile([128, ROWS], fp32, tag="ret")
            nc.tensor.matmul(skp, Wsk_sb[:, 0, :], fTr[:, 0, :],
                             start=True, stop=False)
            nc.tensor.matmul(skp, Wsk_sb[:, 1, :], fTr[:, 1, :],
                             start=False, stop=False)
            nc.tensor.matmul(skp, bsk_sb, ones512, start=False, stop=True,
                             skip_group_check=True)
            skipT = singles.tile([128, ROWS], fp32)
            nc.vector.tensor_copy(skipT, skp)
            s_sb = singles.tile([1, ROWS], fp32)
            nc.vector.tensor_copy(s_sb, s_ps)
            rec = singles.tile([1, ROWS], fp32)
            nc.vector.reciprocal(rec, s_sb)
            rb_ps = psR.tile([128, ROWS], fp32, tag="ret")
            nc.tensor.matmul(rb_ps, ones_f32, rec, start=True, stop=True)
            rb_sb = singles.tile([128, ROWS], fp32)
            nc.scalar.copy(out=rb_sb, in_=rb_ps)
            tmp = singles.tile([128, ROWS], fp32)
            nc.vector.tensor_mul(tmp, outT_ps, rb_sb)
            out_sb = singles.tile([128, ROWS], fp32)
            nc.vector.tensor_add(out_sb, tmp, skipT)
            nc.sync.dma_start(out=out_t.ap(), in_=out_sb)

    nc.compile()
    return nc


def _get_nc():
    if "nc" not in _cache:
        _cache["nc"] = _build()
    return _cache["nc"]


def _quantize_ef_feedback(e_features, w_ae):
    """fp8 e4m3 codes for ef with error feedback across E (desc |w|).

    Returns (codes [B,N,N,E] e4m3, w_hat [E] f32)."""
    import ml_dtypes
    f8 = ml_dtypes.float8_e4m3
    f32 = np.float32
    w = np.asarray(w_ae, f32).reshape(E)
    wh = w.astype(f8).astype(f32)
    order = np.argsort(-np.abs(w))
    ef = np.asarray(e_features, f32)
    codes = np.empty(ef.shape, dtype=f8)
    carry = np.zeros(ef.shape[:-1], f32)
    for e in order:
        x = (ef[..., e] * w[e] + carry) / wh[e]
        qc = x.astype(f8)
        codes[..., e] = qc
        carry = x * wh[e] - qc.astype(f32) * wh[e]
    return codes, wh


def _in_maps(hidden, n_features, e_features, g_features, adj,
             W_m, b_m, W_skip, b_skip, w_a1, b_a1, w_a2, b_a2,
             w_ae, b_ae, w_ag, b_ag):
    import ml_dtypes
    bf16 = ml_dtypes.bfloat16
    f8 = ml_dtypes.float8_e4m3
    f32 = np.float32
    asb = lambda x: np.ascontiguousarray(np.asarray(x).astype(bf16))
    bsum = (np.float32(np.asarray(b_a1).reshape(())) +
            np.float32(np.asarray(b_a2).reshape(())) +
            np.float32(np.asarray(b_ae).reshape(())) +
            np.float32(np.asarray(b_ag).reshape(())))

    codes, wh = _quantize_ef_feedback(e_features, w_ae)

    # Wd[(km2*2+es), s, j, km'] = (km2==km') * w_hat[s*4+es*2+j]
    Wd = np.zeros((64, 2, NS, 2, 64), f32)      # [km2, es, s, j, km']
    for es in range(2):
        for s in range(NS):
            for j in range(2):
                np.fill_diagonal(Wd[:, es, s, j, :], wh[s * 4 + es * 2 + j])
    Wd = Wd.reshape(128, NS, 2, 64).astype(f8)

    w12 = np.stack([np.asarray(w_a1, f32).reshape(2, 128)[0],
                    np.asarray(w_a1, f32).reshape(2, 128)[1],
                    np.asarray(w_a2, f32).reshape(2, 128)[0],
                    np.asarray(w_a2, f32).reshape(2, 128)[1]], axis=1)
    sm = np.zeros((128, 136), f32)
    sm[:, 0:4] = w12
    sm[:, 4] = np.asarray(g_features, f32)[0] * 0  # per-core, filled below
    sm[:, 5:6] = np.asarray(w_ag, f32)
    sm[:, 6] = bsum
    sm[0, 8:136] = (np.asarray(b_skip, f32) + np.asarray(b_m, f32)).reshape(O)
    shared = {
        "Wd": np.ascontiguousarray(Wd),
        "Wm": asb(W_m).reshape(2, 128, O).transpose(1, 0, 2),
        "Wsk": asb(W_skip).reshape(2, 128, O).transpose(1, 0, 2),
    }
    shared["Wm"] = np.ascontiguousarray(shared["Wm"])
    shared["Wsk"] = np.ascontiguousarray(shared["Wsk"])
    maps = []
    for c in range(NCORES):
        b, h = c // 2, c % 2
        rows = slice(h * ROWS, (h + 1) * ROWS)
        m = dict(shared)
        # T[(km2*2+es), kc, kg, s, j, r] = codes[r, kc*128+kg*64+km2, s*4+es*2+j]
        Q = codes[b, rows]                              # [512,1024,16] f8
        Q = Q.reshape(ROWS, KC, KG, 64, NS, 2, 2)       # r,kc,kg,km2,s,es,j
        Q = Q.transpose(3, 5, 1, 2, 4, 6, 0)            # km2,es,kc,kg,s,j,r
        m["T"] = np.ascontiguousarray(Q.reshape(128, KC, KG, NS, 2, ROWS))
        A = np.asarray(adj[b], f32)[rows]               # [512,1024]
        AT = A.T.reshape(KC, KG, 64, ROWS).transpose(2, 0, 1, 3)
        m["adjT"] = np.ascontiguousarray(AT.astype(bf16))
        fk = np.stack([np.asarray(n_features[b], f32).T,
                       np.asarray(hidden[b], f32).T])   # [2,128,1024]
        m["fTk"] = np.ascontiguousarray(
            fk.transpose(1, 0, 2).astype(bf16))
        fr = np.stack([np.asarray(n_features[b], f32)[rows].T,
                       np.asarray(hidden[b], f32)[rows].T])
        m["fTr"] = np.ascontiguousarray(
            fr.transpose(1, 0, 2).astype(bf16))
        smb = sm.copy()
        smb[:, 4] = np.asarray(g_features[b], f32)
        m["sm"] = np.ascontiguousarray(smb.astype(bf16))
        maps.append(m)
    return maps


def kernel(hidden, n_features, e_features, g_features, adj,
           W_m, b_m, W_skip, b_skip, w_a1, b_a1, w_a2, b_a2,
           w_ae, b_ae, w_ag, b_ag):
    from concourse import bass_utils
    nc = _get_nc()
    maps = _in_maps(hidden, n_features, e_features, g_features, adj,
                    W_m, b_m, W_skip, b_skip, w_a1, b_a1, w_a2, b_a2,
                    w_ae, b_ae, w_ag, b_ag)
    res = bass_utils.run_bass_kernel_spmd(nc, maps, core_ids=list(range(NCORES)))
    out = np.empty((B, N, O), np.float32)
    for c in range(NCORES):
        b, h = c // 2, c % 2
        out[b, h * ROWS:(h + 1) * ROWS] = res.results[c]["out"].T
    return out


# revision 36
# speedup vs baseline: 1.0205x; 1.0205x over previous
"""GAT message-passing kernel for Trainium2, 8 NeuronCores — v2.

Problem (hardcoded): B=4, N=1024, H=F=O=G=128, E=16.
  features = concat([n_features, hidden], -1)            [B,N,256]
  values   = features @ W_m + b_m                        [B,N,128]
  logits   = att1 + att2^T + (e_features@w_ae) + att_g   [B,N,N]
  coefs    = softmax(leaky_relu(logits) + (adj-1)*1e9)
  out      = coefs @ values + features @ W_skip + b_skip

Sharding: 8 cores = (batch b = core//2) x (row half = core%2): each core
owns 512 query rows of one batch; keys unsharded. No collectives.

Design (85 us v1 -> ~76-82 us): memory-bound on the e_features stream;
halve its bytes with fp8 and restructure attention so no engine sits on
the DMA critical path.
  - ef is quantized host-side to fp8 e4m3 with ERROR-FEEDBACK across
    the E dim (descending |w_ae|): each channel's quantization residual
    is carried into the next channel, so the on-chip dot product
    sum_e ef_e*w_e lands at bf16-level accuracy (2.7e-3) at half the
    HBM traffic (8 MiB/core).
  - TRANSPOSED attention: logits are built as lps[key, row] per
    128-key chunk. Softmax normalization moves to a ones-matmul
    (PSUM-accumulated across chunks) and A@V consumes the masked
    coefs^T tile directly as the PE moving operand with V stationary —
    no PE transposes, no PSUM->SBUF copies on ACT.
  - The E-contraction runs in DoubleRow fp8 (2 elems/lane/cycle):
    stationary Wd[(km2,es),j,km'] block-diagonal over 64 keys with 4
    channels (es,j) per s-group; out partitions 0..63 (DR requires full
    128-col footprint), so the pipeline works on two [64,512] PSUM
    tiles per chunk.
  - exp(leaky_relu(x)) = max(exp(x+b), exp(.01x+.01b)): two Exp ACT ops
    (ONE act-table set - Lrelu/Exp alternation costs a 1.3us table load
    per op) with att2+att_g+biases on the per-partition bias port; att1
    rides a rank-1 matmul into PSUM. adj mask multiplies post-exp (DVE).
  - Scheduling (the real battle): early-need constants ride the sync
    HWDGE queue AHEAD of the ef stream (SWDGE receipt is ~2us slower and
    13 queued constants starved the HWDGE DMA semaphore lanes, gating ef
    issue); the rest load via SWDGE (separate lane pool). V/att2 are
    produced just-in-time in-loop sharing fTk stationaries; A@V runs at
    pipeline depth 2; a zero-weight matmul spin + heaters (into the
    accumulating PSUM, adding 0) bridge PE idle so the HAM clock-gate
    holds 2.4 GHz.
  - Output is produced transposed [O, rows] and un-transposed on host.
"""

import os
import numpy as np

B, N, H, F, E, G, O = 4, 1024, 128, 128, 16, 128, 128
DIN = F + H
NCORES = 8
ROWS = N // 2          # query rows per core
KC = N // 128          # key chunks of 128
KG = 2                 # 64-key groups per chunk
NS = 4                 # s-groups (4 channels (es,j) each)
NPAIR = KC // 2        # ef DMA tiles (2 chunks each)

_cache = {}


def _build():
    from contextlib import ExitStack
    import concourse.bacc as bacc
    import concourse.tile as tile
    import concourse.mybir as mybir

    fp32 = mybir.dt.float32
    bf16 = mybir.dt.bfloat16
    f8 = mybir.dt.float8e4
    AF = mybir.ActivationFunctionType
    DR = mybir.MatmulPerfMode.DoubleRow

    nc = bacc.Bacc("TRN2", target_bir_lowering=False, debug=False,
                   num_devices=NCORES)

    # T[p=(km2*2+es), kc, kg, s, j, r] fp8 codes of ef
    T_in = nc.dram_tensor("T", [128, KC, KG, NS, 2, ROWS], f8,
                          kind="ExternalInput")
    Wd_in = nc.dram_tensor("Wd", [128, NS, 2, 64], f8, kind="ExternalInput")
    adjT_in = nc.dram_tensor("adjT", [64, KC, KG, ROWS], bf16,
                             kind="ExternalInput")
    fTk_in = nc.dram_tensor("fTk", [128, 2, N], bf16, kind="ExternalInput")
    fTr_in = nc.dram_tensor("fTr", [128, 2, ROWS], bf16, kind="ExternalInput")
    Wm_in = nc.dram_tensor("Wm", [128, 2, O], bf16, kind="ExternalInput")
    Wsk_in = nc.dram_tensor("Wsk", [128, 2, O], bf16, kind="ExternalInput")
    sm_in = nc.dram_tensor("sm", [128, 136], bf16, kind="ExternalInput")
    out_t = nc.dram_tensor("out", [O, ROWS], fp32, kind="ExternalOutput")

    with tile.TileContext(nc) as tc:
        with ExitStack() as ctx:
            singles = ctx.enter_context(tc.tile_pool(name="singles", bufs=1))
            efp = ctx.enter_context(tc.tile_pool(name="efp", bufs=8))
            work = ctx.enter_context(tc.tile_pool(name="work", bufs=4))
            psL = ctx.enter_context(tc.tile_pool(name="psL", bufs=2,
                                                 space="PSUM"))
            psO = ctx.enter_context(tc.tile_pool(name="psO", bufs=1,
                                                 space="PSUM"))
            psS = ctx.enter_context(tc.tile_pool(name="psS", bufs=1,
                                                 space="PSUM"))
            psR = ctx.enter_context(tc.tile_pool(name="psR", bufs=2,
                                                 space="PSUM"))

            # ---- early constants on sync HWDGE (land ~10us), then ef ---
            smalls = singles.tile([128, 136], bf16)
            nc.sync.dma_start(out=smalls, in_=sm_in.ap())
            w12_sb = smalls[:, 0:4]
            g_sb = smalls[:, 4:5]
            wag_sb = smalls[:, 5:6]
            bsb_sb = smalls[:, 6:7]
            bsk_sb = smalls[0:1, 8:136]
            Wd_sb = singles.tile([128, NS, 2, 64], f8)
            nc.sync.dma_start(out=Wd_sb, in_=Wd_in.ap())
            fTr = singles.tile([128, 2, ROWS], bf16)
            nc.sync.dma_start(out=fTr, in_=fTr_in.ap())

            # ---- ef stream (sync HWDGE queue) --------------------------
            ef_tiles = []
            for p in range(KC):
                t = efp.tile([128, KG, NS, 2, ROWS], f8, tag="ef")
                if p < 2:
                    nc.sync.dma_start(out=t[:, 0], in_=T_in[:, p, 0])
                    nc.sync.dma_start(out=t[:, 1], in_=T_in[:, p, 1])
                else:
                    nc.sync.dma_start(out=t, in_=T_in[:, p])
                ef_tiles.append(t)

            # ---- remaining constants (SWDGE queue) ---------------------
            fTk = singles.tile([128, 2, N], bf16)
            nc.gpsimd.dma_start(out=fTk, in_=fTk_in.ap())
            Wm_sb = singles.tile([128, 2, O], bf16)
            nc.gpsimd.dma_start(out=Wm_sb, in_=Wm_in.ap())
            adjT = singles.tile([64, KC, KG, ROWS], bf16)
            nc.gpsimd.dma_start(out=adjT[:, 0:4], in_=adjT_in[:, 0:4])
            nc.gpsimd.dma_start(out=adjT[:, 4:8], in_=adjT_in[:, 4:8])
            Wsk_sb = singles.tile([128, 2, O], bf16)
            nc.gpsimd.dma_start(out=Wsk_sb, in_=Wsk_in.ap())

            ones_bf = singles.tile([1, 128], bf16)
            nc.vector.memset(ones_bf, 1.0)
            ones512 = singles.tile([1, ROWS], bf16)
            nc.vector.memset(ones512, 1.0)
            ones64c = singles.tile([64, 1], bf16)
            nc.vector.memset(ones64c, 1.0)
            ones_f32 = singles.tile([1, 128], fp32)
            nc.vector.memset(ones_f32, 1.0)
            w0 = singles.tile([128, 128], bf16)
            nc.vector.memset(w0, 0.0)
            w0f8 = singles.tile([128, 128], f8)
            nc.vector.memset(w0f8, 0.0)

            # accumulators (also double as warmup/heater scratch: zero-weight
            # matmuls add 0; the first real av() has start=True which clears)
            outT_ps = psO.tile([128, ROWS], fp32, tag="o")
            s_ps = psS.tile([1, ROWS], fp32, tag="s")

            # PE warmup spin: sustained activity flips HAM to 8/8 early
            for i in range(24):
                nc.tensor.matmul(outT_ps[:, 0:128], w0, w0,
                                 start=(i == 0), stop=False,
                                 tile_position=(0, 0), skip_group_check=True)

            # ---- phase 0: V, att2, att1, skipT, att_g ------------------
            # att_g + sum(biases): sc = g@wag + bs -> bcast [64,1]
            gps = psR.tile([1, 1], fp32, tag="ret")
            nc.tensor.matmul(gps, g_sb, wag_sb, start=True, stop=True)
            bsf = singles.tile([1, 1], fp32)
            nc.vector.tensor_copy(bsf, bsb_sb[0:1, :])
            sc1 = singles.tile([1, 1], fp32)
            nc.vector.tensor_copy(sc1, gps)
            nc.vector.tensor_scalar_add(sc1, sc1, bsf)


            # att1 over this core's rows, + (att_g + biases): [1, ROWS]
            a1ps = psR.tile([1, ROWS], fp32, tag="ret")
            nc.tensor.matmul(a1ps, w12_sb[:, 0:1], fTr[:, 0, :],
                             start=True, stop=False)
            nc.tensor.matmul(a1ps, w12_sb[:, 1:2], fTr[:, 1, :],
                             start=False, stop=True)
            a1f = singles.tile([1, ROWS], fp32)
            nc.vector.tensor_scalar_add(a1f, a1ps, sc1)
            att1r = singles.tile([1, ROWS], bf16)
            nc.vector.tensor_copy(att1r, a1f)

            # ---- main loop over key chunks (software-pipelined) --------
            V_sb = singles.tile([64, KC * KG, O], bf16)
            att2b = singles.tile([64, KC * KG], fp32)
            cT_tiles = {}

            def contract(kc):
                Ekc = ef_tiles[kc]          # [128, KG, NS, 2, ROWS]
                lps = []
                for kg in range(KG):
                    lp = psL.tile([64, ROWS], fp32, tag=f"l{kg}")
                    for s in range(NS):
                        nc.tensor.matmul(lp, Wd_sb[:, s], Ekc[:, kg, s],
                                         start=(s == 0), stop=False,
                                         perf_mode=DR, tile_position=(0, 0))
                    nc.tensor.matmul(lp, ones_bf[:, 0:64], att1r,
                                     start=False, stop=True,
                                     skip_group_check=True)
                    lps.append(lp)
                return lps

            def vmm2(p):
                # V[key64, O] + att2[key64] for 4 half-chunks (2 kc) at once
                vps = psR.tile([64, 4, O], fp32, tag="ret")
                aps = psR.tile([64, 4], fp32, tag="ret")
                for i in range(4):
                    idx = p * 4 + i
                    ks = slice(idx * 64, idx * 64 + 64)
                    nc.tensor.matmul(vps[:, i, :], fTk[:, 0, ks],
                                     Wm_sb[:, 0, :], start=True, stop=False,
                                     skip_group_check=True)
                    nc.tensor.matmul(vps[:, i, :], fTk[:, 1, ks],
                                     Wm_sb[:, 1, :], start=False, stop=True,
                                     skip_group_check=True)
                    nc.tensor.matmul(aps[:, i:i + 1], fTk[:, 0, ks],
                                     w12_sb[:, 2:3], start=True, stop=False,
                                     skip_group_check=True)
                    nc.tensor.matmul(aps[:, i:i + 1], fTk[:, 1, ks],
                                     w12_sb[:, 3:4], start=False, stop=True,
                                     skip_group_check=True)
                nc.vector.tensor_copy(V_sb[:, p * 4:p * 4 + 4, :], vps)
                nc.vector.tensor_copy(att2b[:, p * 4:p * 4 + 4], aps)

            def softmax_mask(kc, lps):
                lrl = work.tile([64, KG, ROWS], fp32, tag="lrl")
                ex = work.tile([64, KG, ROWS], bf16, tag="ex")
                for kg in range(KG):
                    idx = kc * KG + kg
                    nc.scalar.activation(lrl[:, kg, :], lps[kg], AF.Lrelu,
                                         bias=att2b[:, idx:idx + 1],
                                         alpha=0.01)
                nc.scalar.activation(ex, lrl, AF.Exp)
                cT = work.tile([64, KG, ROWS], bf16, tag="cT")
                nc.vector.tensor_mul(cT, ex, adjT[:, kc])
                cT_tiles[kc] = cT

            def av(kc):
                cT = cT_tiles.pop(kc)
                for kg in range(KG):
                    idx = kc * KG + kg
                    first = (kc == 0 and kg == 0)
                    last = (kc == KC - 1 and kg == KG - 1)
                    nc.tensor.matmul(s_ps, ones64c, cT[:, kg, :],
                                     start=first, stop=last,
                                     skip_group_check=True)
                    nc.tensor.matmul(outT_ps, V_sb[:, idx, :], cT[:, kg, :],
                                     start=first, stop=last,
                                     skip_group_check=True)

            for kc in range(KC):
                if kc >= 3:
                    av(kc - 3)
                if kc < 4:
                    vmm2(kc)
                if 1 <= kc <= 5:
                    # zero-weight heaters: keep the PE HAM-warm while the
                    # ef stream ramps; numerically add 0 to outT
                    for _ in range(2):
                        nc.tensor.matmul(outT_ps[:, 0:ROWS],
                                         w0f8, ef_tiles[kc - 1][:, 0, 0, 0],
                                         start=False, stop=False,
                                         skip_group_check=True)
                lps = contract(kc)
                softmax_mask(kc, lps)
            av(KC - 3)
            av(KC - 2)
            av(KC - 1)

            # ---- skip^T, normalize, add, store -------------------------
            skp = psR.tile([128, ROWS], fp32, tag="ret")
            nc.tensor.matmul(skp, Wsk_sb[:, 0, :], fTr[:, 0, :],
                             start=True, stop=False)
            nc.tensor.matmul(skp, Wsk_sb[:, 1, :], fTr[:, 1, :],
                             start=False, stop=False)
            nc.tensor.matmul(skp, bsk_sb, ones512, start=False, stop=True,
                             skip_group_check=True)
            skipT = singles.tile([128, ROWS], fp32)
            nc.vector.tensor_copy(skipT, skp)
            s_sb = singles.tile([1, ROWS], fp32)
            nc.vector.tensor_copy(s_sb, s_ps)
            rec = singles.tile([1, ROWS], fp32)
            nc.vector.reciprocal(rec, s_sb)
            rb_ps = psR.tile([128, ROWS], fp32, tag="ret")
            nc.tensor.matmul(rb_ps, ones_f32, rec, start=True, stop=True)
            rb_sb = singles.tile([128, ROWS], fp32)
            nc.scalar.copy(out=rb_sb, in_=rb_ps)
            tmp = singles.tile([128, ROWS], fp32)
            nc.vector.tensor_mul(tmp, outT_ps, rb_sb)
            out_sb = singles.tile([128, ROWS], fp32)
            nc.vector.tensor_add(out_sb, tmp, skipT)
            nc.sync.dma_start(out=out_t.ap(), in_=out_sb)

    nc.compile()
    return nc


def _get_nc():
    if "nc" not in _cache:
        _cache["nc"] = _build()
    return _cache["nc"]


def _quantize_ef_feedback(e_features, w_ae):
    """fp8 e4m3 codes for ef with error feedback across E (desc |w|).

    Returns (codes [B,N,N,E] e4m3, w_hat [E] f32)."""
    import ml_dtypes
    f8 = ml_dtypes.float8_e4m3
    f32 = np.float32
    w = np.asarray(w_ae, f32).reshape(E)
    wh = w.astype(f8).astype(f32)
    order = np.argsort(-np.abs(w))
    ef = np.asarray(e_features, f32)
    codes = np.empty(ef.shape, dtype=f8)
    carry = np.zeros(ef.shape[:-1], f32)
    for e in order:
        x = (ef[..., e] * w[e] + carry) / wh[e]
        qc = x.astype(f8)
        codes[..., e] = qc
        carry = x * wh[e] - qc.astype(f32) * wh[e]
    return codes, wh


def _in_maps(hidden, n_features, e_features, g_features, adj,
             W_m, b_m, W_skip, b_skip, w_a1, b_a1, w_a2, b_a2,
             w_ae, b_ae, w_ag, b_ag):
    import ml_dtypes
    bf16 = ml_dtypes.bfloat16
    f8 = ml_dtypes.float8_e4m3
    f32 = np.float32
    asb = lambda x: np.ascontiguousarray(np.asarray(x).astype(bf16))
    bsum = (np.float32(np.asarray(b_a1).reshape(())) +
            np.float32(np.asarray(b_a2).reshape(())) +
            np.float32(np.asarray(b_ae).reshape(())) +
            np.float32(np.asarray(b_ag).reshape(())))

    codes, wh = _quantize_ef_feedback(e_features, w_ae)

    # Wd[(km2*2+es), s, j, km'] = (km2==km') * w_hat[s*4+es*2+j]
    Wd = np.zeros((64, 2, NS, 2, 64), f32)      # [km2, es, s, j, km']
    for es in range(2):
        for s in range(NS):
            for j in range(2):
                np.fill_diagonal(Wd[:, es, s, j, :], wh[s * 4 + es * 2 + j])
    Wd = Wd.reshape(128, NS, 2, 64).astype(f8)

    w12 = np.stack([np.asarray(w_a1, f32).reshape(2, 128)[0],
                    np.asarray(w_a1, f32).reshape(2, 128)[1],
                    np.asarray(w_a2, f32).reshape(2, 128)[0],
                    np.asarray(w_a2, f32).reshape(2, 128)[1]], axis=1)
    sm = np.zeros((128, 136), f32)
    sm[:, 0:4] = w12
    sm[:, 4] = np.asarray(g_features, f32)[0] * 0  # per-core, filled below
    sm[:, 5:6] = np.asarray(w_ag, f32)
    sm[:, 6] = bsum
    sm[0, 8:136] = (np.asarray(b_skip, f32) + np.asarray(b_m, f32)).reshape(O)
    shared = {
        "Wd": np.ascontiguousarray(Wd),
        "Wm": asb(W_m).reshape(2, 128, O).transpose(1, 0, 2),
        "Wsk": asb(W_skip).reshape(2, 128, O).transpose(1, 0, 2),
    }
    shared["Wm"] = np.ascontiguousarray(shared["Wm"])
    shared["Wsk"] = np.ascontiguousarray(shared["Wsk"])
    maps = []
    for c in range(NCORES):
        b, h = c // 2, c % 2
        rows = slice(h * ROWS, (h + 1) * ROWS)
        m = dict(shared)
        # T[(km2*2+es), kc, kg, s, j, r] = codes[r, kc*128+kg*64+km2, s*4+es*2+j]
        Q = codes[b, rows]                              # [512,1024,16] f8
        Q = Q.reshape(ROWS, KC, KG, 64, NS, 2, 2)       # r,kc,kg,km2,s,es,j
        Q = Q.transpose(3, 5, 1, 2, 4, 6, 0)            # km2,es,kc,kg,s,j,r
        m["T"] = np.ascontiguousarray(Q.reshape(128, KC, KG, NS, 2, ROWS))
        A = np.asarray(adj[b], f32)[rows]               # [512,1024]
        AT = A.T.reshape(KC, KG, 64, ROWS).transpose(2, 0, 1, 3)
        m["adjT"] = np.ascontiguousarray(AT.astype(bf16))
        fk = np.stack([np.asarray(n_features[b], f32).T,
                       np.asarray(hidden[b], f32).T])   # [2,128,1024]
        m["fTk"] = np.ascontiguousarray(
            fk.transpose(1, 0, 2).astype(bf16))
        fr = np.stack([np.asarray(n_features[b], f32)[rows].T,
                       np.asarray(hidden[b], f32)[rows].T])
        m["fTr"] = np.ascontiguousarray(
            fr.transpose(1, 0, 2).astype(bf16))
        smb = sm.copy()
        smb[:, 4] = np.asarray(g_features[b], f32)
        m["sm"] = np.ascontiguousarray(smb.astype(bf16))
        maps.append(m)
    return maps


def kernel(hidden, n_features, e_features, g_features, adj,
           W_m, b_m, W_skip, b_skip, w_a1, b_a1, w_a2, b_a2,
           w_ae, b_ae, w_ag, b_ag):
    from concourse import bass_utils
    nc = _get_nc()
    maps = _in_maps(hidden, n_features, e_features, g_features, adj,
                    W_m, b_m, W_skip, b_skip, w_a1, b_a1, w_a2, b_a2,
                    w_ae, b_ae, w_ag, b_ag)
    res = bass_utils.run_bass_kernel_spmd(nc, maps, core_ids=list(range(NCORES)))
    out = np.empty((B, N, O), np.float32)
    for c in range(NCORES):
        b, h = c // 2, c % 2
        out[b, h * ROWS:(h + 1) * ROWS] = res.results[c]["out"].T
    return out
